# revision 1
# baseline (speedup 1.0000x reference)
"""GAT (nn_GAT_29523605193094) Trainium2 kernel.

The reference keeps the source bug ``src, dst = edges[0], edges[0]``, so the
adjacency matrix is purely diagonal: adj[i, i] = (i appears in edges[0]).
After the -inf masking, row i of the [N, N, H] score tensor has exactly one
finite entry (j = i) when node i is covered, so softmax over axis=1 yields
exactly 1.0 at (i, i) and 0.0 elsewhere, and the output row is exactly
h[i] = (X @ W)[i].  Rows for uncovered nodes are all -inf -> softmax is NaN
-> the output row is NaN.  Both cases are reproduced here bit-exactly:

    out = X @ W            (on 8 NeuronCores, row-sharded)
    out[~covered] = NaN    (host-side mask from edges[0])

The device work is a row-sharded [4096, 512] @ [512, 256] fp32 matmul.
Each core gets 512 rows of X (pre-transposed on host to the [K, M] layout
the PE wants for the stationary operand) plus the full W.
"""

import numpy as np

N = 4096
IN = 512
OUT = 256
NCORES = 8
RB = N // NCORES  # 512 rows per core
P = 128
KT = IN // P      # 4 contraction chunks
MT = RB // P      # 4 output row blocks per core

_state = {}

# test.py reads this after a traced call for the HW exec time.
LAST_RESULTS = None


def _build():
    import concourse.mybir as mybir
    import concourse.tile as tile
    from concourse import bacc
    from concourse.bass import ts

    nc = bacc.Bacc(
        "TRN2",
        target_bir_lowering=False,
        debug=False,
        num_devices=NCORES,
    )
    f32 = mybir.dt.float32
    xt = nc.dram_tensor("xt", [IN, RB], f32, kind="ExternalInput")   # X_shard^T
    w = nc.dram_tensor("w", [IN, OUT], f32, kind="ExternalInput")
    out = nc.dram_tensor("out", [RB, OUT], f32, kind="ExternalOutput")

    with tile.TileContext(nc) as tc:
        with (
            tc.tile_pool(name="ins", bufs=1) as in_pool,
            tc.tile_pool(name="outs", bufs=4) as out_pool,
            tc.tile_pool(name="ps", bufs=4, space="PSUM") as psum_pool,
        ):
            xt_t = in_pool.tile([P, KT, RB], f32)
            w_t = in_pool.tile([P, KT, OUT], f32)
            for k in range(KT):
                nc.sync.dma_start(xt_t[:, k, :], xt[ts(k, P), :])
                nc.sync.dma_start(w_t[:, k, :], w[ts(k, P), :])

            for m in range(MT):
                ps = psum_pool.tile([P, OUT], f32)
                for k in range(KT):
                    nc.tensor.matmul(
                        ps[:],
                        xt_t[:, k, ts(m, P)],
                        w_t[:, k, :],
                        start=(k == 0),
                        stop=(k == KT - 1),
                    )
                ob = out_pool.tile([P, OUT], f32)
                nc.vector.tensor_copy(ob[:], ps[:])
                nc.sync.dma_start(out[ts(m, P), :], ob[:])

    nc.compile()
    return nc


def kernel(X, edges, W, A):
    global LAST_RESULTS
    from concourse.bass_utils import run_bass_kernel_spmd

    X = np.ascontiguousarray(np.asarray(X, dtype=np.float32))
    W = np.ascontiguousarray(np.asarray(W, dtype=np.float32))
    edges = np.asarray(edges)

    if "nc" not in _state:
        _state["nc"] = _build()
    nc = _state["nc"]

    XT = np.ascontiguousarray(X.T)  # [IN, N]
    in_maps = [
        {"xt": np.ascontiguousarray(XT[:, c * RB : (c + 1) * RB]), "w": W}
        for c in range(NCORES)
    ]
    res = run_bass_kernel_spmd(nc, in_maps, core_ids=list(range(NCORES)))
    LAST_RESULTS = res
    out = np.concatenate([res.results[c]["out"] for c in range(NCORES)], axis=0)

    # Reference semantics: nodes absent from edges[0] have an all -inf score
    # row; softmax of that is NaN, which propagates to the output row.
    covered = np.zeros(N, dtype=bool)
    covered[edges[0]] = True
    if not covered.all():
        out[~covered] = np.nan
    return out
